# revision 36
# baseline (speedup 1.0000x reference)
"""Multi-head self-attention (B=4, N=2048, D=1024, H=16) on 8 trn2 NeuronCores.

Sharding: 8 shards = (batch, head-half).  Core c handles batch c//2 and heads
[(c%2)*8, (c%2)*8+8) -- tensor parallel over heads: w_q/w_k/w_v column-sliced
by head, w_o row-sliced; the partial-output all-reduce of the tensor-parallel
unshard is folded into the host-side gather together with the bias add.

Host-side input marshalling (layout only, no FLOPs): z is transposed per
batch to [D, N] and cast to bf16; weight slices are cast to bf16.

Per-core kernel (Tile), all SBUF-resident, software-pipelined so the ACT
exp stream (the phase-2 co-bottleneck) starts ~40us in:
  emit order:  V' proj | K/Q head-pair 0 | head 0 | K/Q pair 1 | head 1 |
               K/Q pair 2 | head 2 | K/Q pair 3 | heads 3-7 |
               q-half 1 heads 0-7 with q-half-0 out-proj chunks interleaved |
               q-half-1 out-proj chunks.
  scores S^T = K Q^T on 128-row zero-padded contraction, exp(s/8) on ACT to
  bf16, P^T @ [V_h | 1] accumulated over 16 key chunks (denominator in row
  64), PSUM freed via a fast copy to SBUF, reciprocal_approx_fast +
  gpsimd partition-broadcast for the normalize, out-proj over the local 512
  attn dims only (partial sums, host combines).
PSUM banks: scores 2x[128,1024]=4, PV 1x[65,1024]=2, projections
2x[128,512]=2 (pool released before the out-proj pool 1x[128,1024]=2 opens).
"""

import os
import sys

_TRN_REPO = "/opt/trn_rl_repo"
if os.path.isdir(_TRN_REPO) and _TRN_REPO not in sys.path:
    sys.path.insert(0, _TRN_REPO)

import ml_dtypes
import numpy as np

import concourse.bass as bass  # noqa: E402
import concourse.mybir as mybir  # noqa: E402
from concourse import bacc  # noqa: E402
from concourse.bass_utils import run_bass_kernel_spmd  # noqa: E402
from concourse.tile import TileContext  # noqa: E402

F32 = mybir.dt.float32
BF16 = mybir.dt.bfloat16
MULT = mybir.AluOpType.mult
EXP = mybir.ActivationFunctionType.Exp

N_CORES = 8
B, N, D = 4, 2048, 1024
H, HD = 16, 64
HL = 8            # heads per core
DH = HL * HD      # 512 local attn dims
P = 128
DC = D // P       # 8 din chunks
HC = DH // P      # 4 local dout chunks (2 heads each)
NKC = N // P      # 16 key chunks
NQH = N // 2      # 1024 queries per half
SCALE = 1.0 / 8.0  # 1/sqrt(HD)
BF = ml_dtypes.bfloat16


def _build():
    nc = bacc.Bacc("TRN2", target_bir_lowering=False, debug=False,
                   num_devices=N_CORES)
    zt_d = nc.declare_dram_parameter("zt", [D, N], BF16, isOutput=False)
    wq_d = nc.declare_dram_parameter("wq", [D, DH], BF16, isOutput=False)
    wk_d = nc.declare_dram_parameter("wk", [D, DH], BF16, isOutput=False)
    wv_d = nc.declare_dram_parameter("wv", [D, DH], BF16, isOutput=False)
    wo_d = nc.declare_dram_parameter("wo", [DH, D], BF16, isOutput=False)
    out_d = nc.declare_dram_parameter("out", [N, D], BF16, isOutput=True)

    with TileContext(nc) as tc:
        pp = tc.alloc_tile_pool(name="persist", bufs=1)
        # Per-head scores operands: head h in partitions 0-63 of slot h,
        # partitions 64-127 zero (full 128-row contraction keeps HAM warm).
        ktp = pp.tile([P, HL, N], BF16)
        qtp = pp.tile([P, HL, N], BF16)
        # V' = [V_h | 1] per head: [keys 128, key-chunk, head, 65] bf16
        vp = pp.tile([P, NKC, HL, HD + 1], BF16)
        nc.vector.memset(vp[:, :, :, HD], 1.0)
        attnT = pp.tile([P, HC, N], BF16)
        wo_sb = pp.tile([P, HC, D], BF16)

        # pools released mid-kernel (p1p/zp/wp) are allocated last so the
        # release order stays LIFO for the tile-pool allocator
        ssp = tc.alloc_tile_pool(name="pss", bufs=2, space="PSUM")
        pvp = tc.alloc_tile_pool(name="pvo", bufs=1, space="PSUM")
        esp = tc.alloc_tile_pool(name="es", bufs=4)
        nrm = tc.alloc_tile_pool(name="nrm", bufs=2)
        outp = tc.alloc_tile_pool(name="ot", bufs=2)

        # input DMAs split across two queues, z chunks first (every
        # projection needs the full z^T contraction, so z gates the start)
        zp = tc.alloc_tile_pool(name="zin", bufs=1)
        wp = tc.alloc_tile_pool(name="wts", bufs=1)
        zt_sb = zp.tile([P, DC, N], BF16)
        wv_sb = wp.tile([P, DC, DH], BF16)
        wk_sb = wp.tile([P, DC, DH], BF16)
        wq_sb = wp.tile([P, DC, DH], BF16)

        def w_half(eng, w_sb, w_d, lo):
            eng.dma_start(
                w_sb[:, lo:lo + 4, :],
                w_d[lo * P:(lo + 4) * P, :].rearrange("(c p) o -> p c o", p=P))

        def z_chunk(eng, dc):
            eng.dma_start(zt_sb[:, dc, :], zt_d[dc * P:(dc + 1) * P, :])

        # interleave so wk/wq (gating the first scores) land early while z
        # streams on both queues
        w_half(nc.gpsimd, wv_sb, wv_d, 0)
        z_chunk(nc.sync, 0)
        z_chunk(nc.gpsimd, 1)
        z_chunk(nc.sync, 2)
        z_chunk(nc.gpsimd, 3)
        w_half(nc.sync, wk_sb, wk_d, 0)
        w_half(nc.gpsimd, wv_sb, wv_d, 4)
        z_chunk(nc.sync, 4)
        z_chunk(nc.gpsimd, 5)
        w_half(nc.sync, wk_sb, wk_d, 4)
        z_chunk(nc.gpsimd, 7)
        z_chunk(nc.sync, 6)
        w_half(nc.gpsimd, wq_sb, wq_d, 0)
        w_half(nc.sync, wq_sb, wq_d, 4)
        # zero rows 64-127 of every K^T/Q^T head slot on the otherwise-idle
        # gpsimd engine (keeps the DVE FIFO clear for the psum-ring copies);
        # wo is not needed until the q-half-1 out-projection, load it last
        for j in range(HL):
            nc.gpsimd.memset(ktp[64:P, j, :], 0.0)
            nc.gpsimd.memset(qtp[64:P, j, :], 0.0)
        nc.gpsimd.dma_start(wo_sb[:], wo_d.rearrange("(c p) o -> p c o", p=P))

        p1p = tc.alloc_tile_pool(name="psp1", bufs=2, space="PSUM")

        def v_chunk(kc):
            ps = p1p.tile([P, DH], F32, name="p1")
            for dc in range(DC):
                nc.tensor.matmul(
                    ps[:],
                    lhsT=zt_sb[:, dc, kc * P:(kc + 1) * P],
                    rhs=wv_sb[:, dc, :],
                    start=(dc == 0), stop=(dc == DC - 1))
            nc.vector.tensor_copy(
                vp[:, kc, :, 0:HD], ps.rearrange("p (h d) -> p h d", d=HD))

        def kq_chunk(w_sb, dst, oc, sh):
            # [dout 128 (2 heads), 1024 seq] in two 512 psum tiles;
            # dc-outer / q-inner so each LDWEIGHTS serves 2 matmuls.
            ps = [p1p.tile([P, 512], F32, name="p1") for _ in range(2)]
            for dc in range(DC):
                for q2 in range(2):
                    nc.tensor.matmul(
                        ps[q2][:],
                        lhsT=w_sb[:, dc, oc * P:(oc + 1) * P],
                        rhs=zt_sb[:, dc,
                                  sh * 1024 + q2 * 512:sh * 1024 + (q2 + 1) * 512],
                        start=(dc == 0), stop=(dc == DC - 1))
            for q2 in range(2):
                s0 = sh * 1024 + q2 * 512
                nc.vector.tensor_copy(dst[0:64, 2 * oc, s0:s0 + 512],
                                      ps[q2][0:64, :])
                nc.vector.tensor_copy(dst[0:64, 2 * oc + 1, s0:s0 + 512],
                                      ps[q2][64:P, :])

        def head(h, qh, fill=()):
            # fill: dict kc -> thunk, emitted after iteration kc's exp so
            # other PE work spreads through the ACT-bound attention loop
            q0 = qh * NQH
            fill = {kc: list(ts) for kc, ts in dict(fill).items()}
            pso = pvp.tile([HD + 1, NQH], F32, name="pvo")

            def pv_mm(kc, es):
                lh = vp[:, kc, h, :]
                for qc in range(2):
                    nc.tensor.matmul(
                        pso[:, qc * 512:(qc + 1) * 512],
                        lhsT=lh,
                        rhs=es[:, qc * 512:(qc + 1) * 512],
                        start=(kc == 0), stop=(kc == NKC - 1))

            # PV skewed one key-chunk behind scores so the PE never waits
            # on the previous head's PV-psum handoff copy.
            prev_es = None
            for kc in range(NKC):
                ps = ssp.tile([P, NQH], F32, name="pss")
                es = esp.tile([P, NQH], BF16)
                for qc in range(2):
                    nc.tensor.matmul(
                        ps[:, qc * 512:(qc + 1) * 512],
                        lhsT=ktp[:, h, kc * P:(kc + 1) * P],
                        rhs=qtp[:, h, q0 + qc * 512:q0 + (qc + 1) * 512])
                nc.scalar.activation(es[:], ps[:], EXP, scale=SCALE)
                for t in fill.pop(kc, ()):
                    t()
                if prev_es is not None:
                    pv_mm(kc - 1, prev_es)
                prev_es = es
            pv_mm(NKC - 1, prev_es)
            # free the PV psum fast (two copies), normalize off-path.
            # denominator goes to a partition-0 tile: reciprocal_approx_fast
            # miscomputes on partition-offset inputs.
            po = nrm.tile([HD, NQH], F32, tag="po")
            nc.vector.tensor_copy(po[:], pso[0:HD, :])
            den = nrm.tile([1, NQH], F32, tag="den", bufs=1)
            nc.vector.tensor_copy(den[:], pso[HD:HD + 1, :])
            rec = nrm.tile([1, NQH], F32, tag="rec", bufs=1)
            nc.vector.reciprocal_approx_fast(out=rec[:], in_=den[:])
            rb = nrm.tile([64, NQH], F32, tag="rb", bufs=1)
            nc.gpsimd.partition_broadcast(rb[:], rec[:])
            pr = 64 * (h % 2)
            nc.vector.tensor_tensor(
                attnT[pr:pr + 64, h // 2, q0:q0 + NQH],
                po[:], rb[:], MULT)

        # ---- emission: K/Q sh0 of pair 0 first (all head 0's early scores
        # need), then the attention loop with all remaining PE work (V, the
        # sh1 halves, K/Q pairs 1-3, out-proj) spread through the ACT-bound
        # head iterations so neither engine starves ----
        # the first V chunks fill the PE while the K/Q weight halves land
        for kc in range(6):
            v_chunk(kc)
        kq_chunk(wk_sb, ktp, 0, 0)
        kq_chunk(wq_sb, qtp, 0, 0)

        # head 0 carries the rest of the V projection (PV is skewed one kc
        # behind, so v_chunk(kc) lands just before pv_mm(kc) needs it) +
        # the pair-0 sh1 halves
        fill0 = {kc: [lambda kc=kc: v_chunk(kc)] for kc in range(6, NKC)}
        fill0[2] = [lambda: kq_chunk(wk_sb, ktp, 0, 1)]
        fill0[4] = [lambda: kq_chunk(wq_sb, qtp, 0, 1)]
        head(0, 0, fill=fill0)
        # heads 1-5 carry K/Q pairs 1-3 (pair i must precede head 2i)
        spread = {1: [(1, 0, 0), (1, 0, 1), (1, 1, 0), (1, 1, 1)],
                  2: [(2, 0, 0), (2, 0, 1)], 3: [(2, 1, 0), (2, 1, 1)],
                  4: [(3, 0, 0), (3, 0, 1)], 5: [(3, 1, 0), (3, 1, 1)]}
        for h in range(1, HL):
            fill = {}
            for i, (oc, w2, sh) in enumerate(spread.get(h, [])):
                w_sb, dst = ((wk_sb, ktp), (wq_sb, qtp))[w2]
                step = NKC // len(spread[h])
                fill[i * step + step - 1] = [
                    lambda w_sb=w_sb, dst=dst, oc=oc, sh=sh:
                    kq_chunk(w_sb, dst, oc, sh)]
            head(h, 0, fill=fill)
            if h == 6:
                p1p.release()
                wp.release()
                zp.release()
        pof = tc.alloc_tile_pool(name="pof", bufs=1, space="PSUM")

        def outproj_half(psf, q8, oc2):
            for dc in range(HC):
                nc.tensor.matmul(
                    psf[:, oc2 * 512:(oc2 + 1) * 512],
                    lhsT=attnT[:, dc, q8 * P:(q8 + 1) * P],
                    rhs=wo_sb[:, dc, oc2 * 512:(oc2 + 1) * 512],
                    start=(dc == 0), stop=(dc == HC - 1))

        def outproj_finish(psf, q8):
            ot = outp.tile([P, D], BF16)
            nc.vector.tensor_copy(ot[:], psf[:])
            nc.sync.dma_start(out_d[q8 * P:(q8 + 1) * P, :], ot[:])

        def outproj_chunk(q8, pool=None):
            # partial out for queries [q8*128, (q8+1)*128) over the local
            # 512 attn dims only; host sums the two partials + bias.
            psf = (pool or pof).tile([P, D], F32, name="pof", tag="pss")
            outproj_half(psf, q8, 0)
            outproj_half(psf, q8, 1)
            outproj_finish(psf, q8)

        # each qh1 head h<7 carries one qh0 out-proj chunk, split into two
        # half-bursts so the PE insertions stay under the ACT lookahead;
        # the last head carries none so the tail starts as early as possible
        def opc_fill(q8):
            psf = [None]

            def first():
                psf[0] = pof.tile([P, D], F32, name="pof", tag="pss")
                outproj_half(psf[0], q8, 0)

            def second():
                outproj_half(psf[0], q8, 1)
                outproj_finish(psf[0], q8)
            return {5: [first], 11: [second]}

        for h in range(HL):
            fill = {}
            if h < 7:
                fill = opc_fill(h)
            if h == 6:
                # chunk 7 rides along after chunk 6 completes (pof bufs=1)
                for kc, ts in zip((13, 15), opc_fill(7).values()):
                    fill.setdefault(kc, []).extend(ts)
            head(h, 1, fill=fill)
        # tail: alternate with the (now idle) scores pool to pipeline
        for q8 in range(8, 16):
            outproj_chunk(q8, pool=(ssp if q8 % 2 else pof))

        for pool in (pof, outp, nrm, esp, pvp, ssp, pp):
            pool.release()

    nc.compile()
    return nc


_NC_CACHE = None


def _get_nc():
    global _NC_CACHE
    if _NC_CACHE is None:
        _NC_CACHE = _build()
    return _NC_CACHE


def _run(z, w_q, w_k, w_v, w_o, b_o, **spmd_kwargs):
    z = np.asarray(z, dtype=np.float32)
    w_q = np.asarray(w_q, dtype=np.float32)
    w_k = np.asarray(w_k, dtype=np.float32)
    w_v = np.asarray(w_v, dtype=np.float32)
    w_o = np.asarray(w_o, dtype=np.float32)
    b_o = np.asarray(b_o, dtype=np.float32)
    assert z.shape == (B, N, D)

    if not spmd_kwargs.get("trace"):
        # A stray BASS_TRACE in the environment would route through the NTFF
        # hook (absent in this image) and crash; force the no-trace path.
        os.environ["BASS_NEVER_TRACE"] = "1"

    nc = _get_nc()
    zt = [np.ascontiguousarray(z[b].T).astype(BF) for b in range(B)]
    wq_h = [np.ascontiguousarray(w_q[:, g * DH:(g + 1) * DH].astype(BF))
            for g in range(2)]
    wk_h = [np.ascontiguousarray(w_k[:, g * DH:(g + 1) * DH].astype(BF))
            for g in range(2)]
    wv_h = [np.ascontiguousarray(w_v[:, g * DH:(g + 1) * DH].astype(BF))
            for g in range(2)]
    wo_h = [np.ascontiguousarray(w_o[g * DH:(g + 1) * DH, :].astype(BF))
            for g in range(2)]
    in_maps = []
    for c in range(N_CORES):
        b, g = c // 2, c % 2
        in_maps.append({"zt": zt[b], "wq": wq_h[g], "wk": wk_h[g],
                        "wv": wv_h[g], "wo": wo_h[g]})

    res = run_bass_kernel_spmd(nc, in_maps, core_ids=list(range(N_CORES)),
                               **spmd_kwargs)
    out = np.empty((B, N, D), dtype=np.float32)
    for b in range(B):
        out[b] = res.results[2 * b]["out"].astype(np.float32)
        out[b] += res.results[2 * b + 1]["out"].astype(np.float32)
        out[b] += b_o[None, :]
    return out, res


def kernel(z, w_q, w_k, w_v, w_o, b_o):
    out, _ = _run(z, w_q, w_k, w_v, w_o, b_o)
    return out
